# revision 28
# baseline (speedup 1.0000x reference)
"""MoE layer (E=8 experts, top-2 routing) on 8 Trainium2 NeuronCores.

Strategy (expert-parallel, per the sharding hint):
  - The gate (T x D @ D x E, softmax, top-2, renorm) is computed on the host
    in fp32; it is ~0.01% of the FLOPs.
  - Tokens are dispatched by expert id ("all-to-all" done host-side): core e
    receives the tokens routed to expert e (padded to a common capacity C),
    together with expert e's weights in bf16.
  - Each core runs a Bass/Tile kernel computing
        y = combine_weight * (gelu(x @ w1 + b1) @ w2 + b2)
    with bf16 matmuls (fp32 PSUM accumulation) on the PE array:
      * phase H: H^T tiles (feature-major) = w1-chunk^T.T @ x^T-chunk,
        so no on-device transposes are needed (w1 natural layout is lhsT).
      * phase Y: token-major Y = H^T-chunk.T @ w2-chunk, which makes the
        per-token combine weight a per-partition scalar.
  - Host "unshard" is two gathers + an add (each token has exactly 2 slots).
"""

import sys
import types

import numpy as np
import ml_dtypes

import concourse.bass as bass
import concourse.mybir as mybir
from concourse import bacc
from concourse.tile import TileContext
from concourse.bass_utils import run_bass_kernel_spmd


def _ensure_antenv_hooks():
    """bass_utils imports antenv.axon_hooks when BASS_TRACE is set; this image
    may lack it. Provide the registry (with the real ctypes NTFF hook when
    available) so tracing works instead of crashing."""
    try:
        import antenv.axon_hooks  # noqa: F401
        return
    except ImportError:
        pass
    if "antenv" not in sys.modules:
        try:
            import antenv  # noqa: F401
        except ImportError:
            sys.modules["antenv"] = types.ModuleType("antenv")
    hooks = types.ModuleType("antenv.axon_hooks")
    state = {"hook": None}
    hooks.set_axon_ntff_profile_hook = lambda h: state.__setitem__("hook", h)
    hooks.get_axon_ntff_profile_hook = lambda: state["hook"]
    sys.modules["antenv"].axon_hooks = hooks
    sys.modules["antenv.axon_hooks"] = hooks
    try:
        from trn_agent_boot.trn_boot import _ntff_profile_via_ctypes
        hook = _ntff_profile_via_ctypes("/opt/axon/libaxon_pjrt.so")
        if hook is not None:
            hooks.set_axon_ntff_profile_hook(hook)
    except Exception:
        pass


_ensure_antenv_hooks()

P = 128
D = 1024
F = 4096
E = 8
TOPK = 2
NBLK = 512

_BF16 = ml_dtypes.bfloat16

_nc_cache: dict = {}
LAST = None  # BassKernelResults of the most recent run (for test harness)


def _build_moe_core(C: int) -> bass.Bass:
    """One-core SPMD program: FFN for C tokens with resident bf16 weights."""
    dt = mybir.dt
    nc = bacc.Bacc("TRN2", target_bir_lowering=False, debug=False)
    KO = D // P    # 8 contraction chunks for x @ w1
    FO = F // P    # 32 contraction chunks for h @ w2
    DN = D // NBLK  # 2 output-column blocks of w2
    GELU = mybir.ActivationFunctionType.Gelu

    # x host-pretiled to the exact SBUF image [block, half, p, ko_local*tok]:
    # each xa/xb DMA is 128 contiguous 4KB lines (vs 512 1KB lines from a
    # [D, C] layout) — fewer descriptors and better HBM bursts during the
    # contended startup window.
    NB = (C + NBLK - 1) // NBLK
    KHALF = (D // P) // 2
    xt = nc.dram_tensor("xt", [NB, 2, P, KHALF, NBLK], dt.bfloat16,
                        kind="ExternalInput")
    # w1 host-pretiled per-fo: w1t[fo, p, ko, j] = w1[ko*P+p, fo*P+j], so each
    # 256KB fo-tile is one contiguous-per-partition DMA and the PE can start
    # after the first tile instead of the full 8MB.
    w1t = nc.dram_tensor("w1t", [FO, P, KO, P], dt.bfloat16,
                         kind="ExternalInput")
    w2 = nc.dram_tensor("w2", [F, D], dt.bfloat16, kind="ExternalInput")
    # b1/sc pre-packed partition-major on host so each DMA is one contiguous
    # descriptor per partition (the rearranged 1-D loads were 4B-strided).
    b1p = nc.dram_tensor("b1p", [P, FO], dt.float32, kind="ExternalInput")
    b2r = nc.dram_tensor("b2r", [P, D], dt.float32, kind="ExternalInput")
    scp = nc.dram_tensor("scp", [P, C // P], dt.float32, kind="ExternalInput")
    # y in bf16: halves the output DMA (tail latency) at ~1e-3 extra rel err.
    y = nc.dram_tensor("y", [C, D], dt.bfloat16, kind="ExternalOutput")

    # Uniform 512-token blocks: smaller N makes the per-matmul LDWEIGHTS
    # (~97ns, FWL off in this toolchain) stop hiding inside the matmul
    # streaming window, measured +94ns/matmul at N=256.
    blocks = []
    off = 0
    while off < C:
        size = min(NBLK, C - off)
        blocks.append((off, size))
        off += size

    with TileContext(nc) as tc:
        with (
            tc.tile_pool(name="w", bufs=1) as wpool,
            tc.tile_pool(name="xin", bufs=2) as xpool,
            tc.tile_pool(name="h", bufs=1) as hpool,
            tc.tile_pool(name="yout", bufs=2) as ypool,
            tc.tile_pool(name="ph", bufs=3, space="PSUM") as phpool,
            tc.tile_pool(name="py", bufs=4, space="PSUM") as pypool,
            tc.tile_pool(name="pw", bufs=1, space="PSUM") as pwpool,
        ):
            KH = KO // 2  # x blocks load as two half-tiles (finer DMA deps)

            def load_x_block(bi):
                xa = xpool.tile([P, KH, NBLK], dt.bfloat16, tag="xa")
                nc.sync.dma_start(xa[:], xt[bi, 0])
                xb = xpool.tile([P, KH, NBLK], dt.bfloat16, tag="xb")
                nc.sync.dma_start(xb[:], xt[bi, 1])
                return xa, xb

            def x_chunk(xts, ko):
                return xts[0][:, ko, :] if ko < KH else xts[1][:, ko - KH, :]

            # Warm the PE's HAM clock gate during the startup DMA window
            # with dummy matmuls on zeroed SBUF. The clock ramp finishes
            # after ~6-8 matmuls; the real stream is gated by xa+w1fo0
            # arrival (~13us under 8-core HBM contention), so 13 warmups
            # (~4us from ~8.3us) cover the ramp and most of the DMA window
            # without pushing the real stream past data-ready.
            warm = wpool.tile([P, NBLK], dt.bfloat16, tag="warm")
            nc.gpsimd.memset(warm[:], 0.0)
            pwarm = pwpool.tile([P, NBLK], dt.float32, tag="pw")
            NWARM = 13
            for i in range(NWARM):
                nc.tensor.matmul(
                    pwarm[:], warm[:, :P], warm[:],
                    start=(i == 0), stop=(i == NWARM - 1),
                )

            # DMA issue order is the startup critical path (each issue costs
            # ~0.6us on the sync queue): xa and w1-fo0 gate the first matmul,
            # xb is needed 4 matmuls later. b1 (tiny, needed at the first
            # gelu ~1.7us into the stream) issues from the idle gpsimd queue
            # so it doesn't take a slot in the critical sync sequence.
            xa0 = xpool.tile([P, KH, NBLK], dt.bfloat16, tag="xa")
            nc.sync.dma_start(xa0[:], xt[0, 0])

            w1sb = []
            t_ = wpool.tile([P, KO, P], dt.bfloat16, tag="w1_0")
            nc.sync.dma_start(t_[:], w1t[0])
            w1sb.append(t_)

            xb0 = xpool.tile([P, KH, NBLK], dt.bfloat16, tag="xb")
            nc.sync.dma_start(xb0[:], xt[0, 1])
            xts0 = (xa0, xb0)

            b1sb = wpool.tile([P, FO], dt.float32, tag="b1")
            nc.gpsimd.dma_start(b1sb[:], b1p[:])

            for fo in range(1, FO):
                t_ = wpool.tile([P, KO, P], dt.bfloat16, tag=f"w1_{fo}")
                nc.sync.dma_start(t_[:], w1t[fo])
                w1sb.append(t_)

            b2sb = wpool.tile([P, D], dt.float32, tag="b2")
            nc.sync.dma_start(b2sb[:], b2r[:])
            scsb = wpool.tile([P, C // P], dt.float32, tag="sc")
            nc.sync.dma_start(scsb[:], scp[:])

            # w2 in 4 chunks: a single 8MB DMA's completion gated the whole
            # Y phase (all 8 cores contend for HBM in the first ~70us and a
            # late w2 stalls the PE ~5us + re-throttles the clock). Chunk 0
            # (2MB) lands ~30us before the first Y matmul needs it, and later
            # chunks arrive while Y streams earlier fo's.
            W2C = 4
            FOC = FO // W2C
            w2_r = w2.rearrange("(fo p) d -> p fo d", p=P)
            w2sb = []
            for c in range(W2C):
                t_ = wpool.tile([P, FOC, D], dt.bfloat16, tag=f"w2_{c}")
                nc.sync.dma_start(t_[:], w2_r[:, c * FOC:(c + 1) * FOC])
                w2sb.append(t_)

            for bi, (n_off, n_size) in enumerate(blocks):
                xts = xts0 if bi == 0 else load_x_block(bi)

                # H^T[f, t] = sum_d w1[d, f] * x^T[d, t], then gelu(+b1).
                htile = hpool.tile([P, FO, NBLK], dt.bfloat16, tag="h")
                for fo in range(FO):
                    ph = phpool.tile([P, NBLK], dt.float32, tag="ph")
                    for ko in range(KO):
                        nc.tensor.matmul(
                            ph[:, :n_size],
                            w1sb[fo][:, ko, :],
                            x_chunk(xts, ko)[:, :n_size],
                            start=(ko == 0),
                            stop=(ko == KO - 1),
                        )
                    nc.scalar.activation(
                        htile[:, fo, :n_size], ph[:, :n_size], GELU,
                        bias=b1sb[:, fo:fo + 1], scale=1.0,
                    )

                # Y[t, d] = sum_f H[t, f] * w2[f, d]; scale per token.
                for tb in range(n_size // P):
                    tbg = (n_off + tb * P) // P
                    ytile = ypool.tile([P, D], dt.bfloat16, tag="y")
                    # dn-outer: the d-half 0 epilogue (scale, store) overlaps
                    # the d-half 1 matmuls, so little epilogue trails the
                    # very last matmul of the kernel.
                    for dn in range(DN):
                        dsl = slice(dn * NBLK, (dn + 1) * NBLK)
                        py = pypool.tile([P, NBLK], dt.float32, tag="py")
                        # Seed PSUM with b2 (vector write), accumulate
                        # matmuls on top with start=False: removes the bias
                        # add from the post-matmul critical path.
                        nc.vector.tensor_copy(py[:], b2sb[:, dsl])
                        for fo in range(FO):
                            nc.tensor.matmul(
                                py[:],
                                htile[:, fo, tb * P:(tb + 1) * P],
                                w2sb[fo // FOC][:, fo % FOC,
                                                dn * NBLK:(dn + 1) * NBLK],
                                start=False,
                                stop=(fo == FO - 1),
                                # start=False with a DVE-seeded bank: the
                                # sim's group bookkeeping never sees a start.
                                skip_group_check=True,
                            )
                        rows = slice(n_off + tb * P, n_off + (tb + 1) * P)
                        nc.vector.tensor_scalar_mul(
                            ytile[:, dsl], py[:], scsb[:, tbg:tbg + 1]
                        )
                        nc.sync.dma_start(y[rows, dsl], ytile[:, dsl])
    nc.compile()
    return nc


def _route(flat, gate_w, gate_b):
    """fp32 gate matching the reference: softmax, top-2, renormalize."""
    logits = flat @ gate_w + gate_b
    m = logits.max(axis=1, keepdims=True)
    p = np.exp(logits - m, dtype=np.float32)
    probs = p / p.sum(axis=1, keepdims=True)
    ti = np.argsort(-probs, axis=1, kind="stable")[:, :TOPK]
    tp = np.take_along_axis(probs, ti, axis=1)
    sw = tp / (tp.sum(axis=1, keepdims=True) + np.float32(1e-9))
    return ti.astype(np.int64), sw.astype(np.float32)


def _dispatch(ti):
    """Slot assignment: (token, k) pair -> (expert, position-in-expert)."""
    Tn = ti.shape[0]
    flat_e = ti.ravel()
    order = np.argsort(flat_e, kind="stable")
    cnt = np.bincount(flat_e, minlength=E)
    starts = np.concatenate([[0], np.cumsum(cnt)[:-1]])
    ranks = np.arange(Tn * TOPK) - starts[flat_e[order]]
    pos = np.empty(Tn * TOPK, np.int64)
    pos[order] = ranks
    return flat_e, pos, cnt, starts, order


def _gelu_exact(v):
    try:
        from scipy.special import erf
        return 0.5 * v * (1.0 + erf(v / np.sqrt(2.0)))
    except ImportError:  # tanh approximation fallback (overflow tokens only)
        return 0.5 * v * (1.0 + np.tanh(
            0.7978845608028654 * (v + 0.044715 * v ** 3)))


def kernel(**inputs) -> np.ndarray:
    global LAST
    x = np.asarray(inputs["x"], np.float32)
    gate_w = np.asarray(inputs["gate_w"], np.float32)
    gate_b = np.asarray(inputs["gate_b"], np.float32)
    w1 = np.asarray(inputs["w1"], np.float32)
    b1 = np.asarray(inputs["b1"], np.float32)
    w2 = np.asarray(inputs["w2"], np.float32)
    b2 = np.asarray(inputs["b2"], np.float32)

    B, S, D_ = x.shape
    flat = x.reshape(-1, D_)
    Tn = flat.shape[0]

    ti, sw = _route(flat, gate_w, gate_b)
    flat_e, pos, cnt, starts, order = _dispatch(ti)

    # Capacity factor 1.0: each core processes exactly T*K/E token slots (the
    # SPMD program is uniform, so every core pays the max expert's cost —
    # capping at the mean keeps the device critical path balanced). The few
    # overflow tokens of the hottest experts are combined on the host in fp32.
    cap = (Tn * TOPK // E + P - 1) // P * P
    C = ((int(cnt.max()) + P - 1) // P) * P
    C = max(min(C, cap), P)

    xT_bf = np.ascontiguousarray(flat.T).astype(_BF16)  # [D, T]
    sw_flat = sw.ravel()

    in_maps = []
    overflow = []
    for e in range(E):
        pairs_all = order[starts[e]:starts[e] + cnt[e]]
        pairs = pairs_all[:C]
        if cnt[e] > C:
            overflow.append((e, pairs_all[C:]))
        n_e = len(pairs)
        toks = pairs // TOPK
        xt_e = np.zeros((D, C), _BF16)
        xt_e[:, :n_e] = xT_bf[:, toks]
        # Pre-tile to the SBUF image [block, half, p, ko_local, tok] so each
        # x DMA is one contiguous 4KB line per partition.
        NB = C // NBLK
        KH = (D // P) // 2
        xt_tiled = np.ascontiguousarray(
            xt_e.reshape(2, KH, P, NB, NBLK).transpose(3, 0, 2, 1, 4)
        )
        sc_e = np.zeros((C,), np.float32)
        sc_e[:n_e] = sw_flat[pairs]
        KO, FO = D // P, F // P
        w1_tiled = np.ascontiguousarray(
            w1[e].astype(_BF16).reshape(KO, P, FO, P).transpose(2, 1, 0, 3)
        )
        in_maps.append({
            "xt": xt_tiled,
            "w1t": w1_tiled,
            "w2": w2[e].astype(_BF16),
            "b1p": np.ascontiguousarray(b1[e].reshape(F // P, P).T),
            "b2r": np.ascontiguousarray(
                np.broadcast_to(b2[e], (P, D))
            ).astype(np.float32),
            "scp": np.ascontiguousarray(sc_e.reshape(C // P, P).T),
        })

    nc = _nc_cache.get(C)
    if nc is None:
        nc = _build_moe_core(C)
        _nc_cache[C] = nc

    LAST = run_bass_kernel_spmd(nc, in_maps, core_ids=list(range(E)))
    Yall = np.stack([
        np.asarray(LAST.results[i]["y"]).astype(np.float32) for i in range(E)
    ])

    # Combine: device slots via two gathers; host fp32 FFN for overflow.
    in_cap = pos < C
    contrib = np.zeros((Tn * TOPK, D_), np.float32)
    idx = np.nonzero(in_cap)[0]
    contrib[idx] = Yall[flat_e[idx], pos[idx]]
    out = contrib[0::TOPK] + contrib[1::TOPK]
    for e, over in overflow:
        toks = over // TOPK
        h = _gelu_exact(flat[toks] @ w1[e] + b1[e])
        y_e = h @ w2[e] + b2[e]
        out[toks] += sw_flat[over][:, None] * y_e
    return out.reshape(B, S, D_).astype(np.float32)



# revision 30
# speedup vs baseline: 1.0070x; 1.0070x over previous
"""MoE layer (E=8 experts, top-2 routing) on 8 Trainium2 NeuronCores.

Strategy (expert-parallel, per the sharding hint):
  - The gate (T x D @ D x E, softmax, top-2, renorm) is computed on the host
    in fp32; it is ~0.01% of the FLOPs.
  - Tokens are dispatched by expert id ("all-to-all" done host-side): core e
    receives the tokens routed to expert e (padded to a common capacity C),
    together with expert e's weights in bf16.
  - Each core runs a Bass/Tile kernel computing
        y = combine_weight * (gelu(x @ w1 + b1) @ w2 + b2)
    with bf16 matmuls (fp32 PSUM accumulation) on the PE array:
      * phase H: H^T tiles (feature-major) = w1-chunk^T.T @ x^T-chunk,
        so no on-device transposes are needed (w1 natural layout is lhsT).
      * phase Y: token-major Y = H^T-chunk.T @ w2-chunk, which makes the
        per-token combine weight a per-partition scalar.
  - Host "unshard" is two gathers + an add (each token has exactly 2 slots).
"""

import sys
import types

import numpy as np
import ml_dtypes

import concourse.bass as bass
import concourse.mybir as mybir
from concourse import bacc
from concourse.tile import TileContext
from concourse.bass_utils import run_bass_kernel_spmd


def _ensure_antenv_hooks():
    """bass_utils imports antenv.axon_hooks when BASS_TRACE is set; this image
    may lack it. Provide the registry (with the real ctypes NTFF hook when
    available) so tracing works instead of crashing."""
    try:
        import antenv.axon_hooks  # noqa: F401
        return
    except ImportError:
        pass
    if "antenv" not in sys.modules:
        try:
            import antenv  # noqa: F401
        except ImportError:
            sys.modules["antenv"] = types.ModuleType("antenv")
    hooks = types.ModuleType("antenv.axon_hooks")
    state = {"hook": None}
    hooks.set_axon_ntff_profile_hook = lambda h: state.__setitem__("hook", h)
    hooks.get_axon_ntff_profile_hook = lambda: state["hook"]
    sys.modules["antenv"].axon_hooks = hooks
    sys.modules["antenv.axon_hooks"] = hooks
    try:
        from trn_agent_boot.trn_boot import _ntff_profile_via_ctypes
        hook = _ntff_profile_via_ctypes("/opt/axon/libaxon_pjrt.so")
        if hook is not None:
            hooks.set_axon_ntff_profile_hook(hook)
    except Exception:
        pass


_ensure_antenv_hooks()

P = 128
D = 1024
F = 4096
E = 8
TOPK = 2
NBLK = 512

_BF16 = ml_dtypes.bfloat16

_nc_cache: dict = {}
LAST = None  # BassKernelResults of the most recent run (for test harness)


def _build_moe_core(C: int) -> bass.Bass:
    """One-core SPMD program: FFN for C tokens with resident bf16 weights."""
    dt = mybir.dt
    nc = bacc.Bacc("TRN2", target_bir_lowering=False, debug=False)
    KO = D // P    # 8 contraction chunks for x @ w1
    FO = F // P    # 32 contraction chunks for h @ w2
    DN = D // NBLK  # 2 output-column blocks of w2
    GELU = mybir.ActivationFunctionType.Gelu

    # x host-pretiled to the exact SBUF image [block, half, p, ko_local*tok]:
    # each xa/xb DMA is 128 contiguous 4KB lines (vs 512 1KB lines from a
    # [D, C] layout) — fewer descriptors and better HBM bursts during the
    # contended startup window.
    NB = (C + NBLK - 1) // NBLK
    KHALF = (D // P) // 2
    xt = nc.dram_tensor("xt", [NB, 2, P, KHALF, NBLK], dt.bfloat16,
                        kind="ExternalInput")
    # w1 host-pretiled per-fo: w1t[fo, p, ko, j] = w1[ko*P+p, fo*P+j], so each
    # 256KB fo-tile is one contiguous-per-partition DMA and the PE can start
    # after the first tile instead of the full 8MB.
    w1t = nc.dram_tensor("w1t", [FO, P, KO, P], dt.bfloat16,
                         kind="ExternalInput")
    w2 = nc.dram_tensor("w2", [F, D], dt.bfloat16, kind="ExternalInput")
    # b1/sc pre-packed partition-major on host so each DMA is one contiguous
    # descriptor per partition (the rearranged 1-D loads were 4B-strided).
    b1p = nc.dram_tensor("b1p", [P, FO], dt.float32, kind="ExternalInput")
    b2r = nc.dram_tensor("b2r", [P, D], dt.float32, kind="ExternalInput")
    scp = nc.dram_tensor("scp", [P, C // P], dt.float32, kind="ExternalInput")
    # y in bf16: halves the output DMA (tail latency) at ~1e-3 extra rel err.
    y = nc.dram_tensor("y", [C, D], dt.bfloat16, kind="ExternalOutput")

    # Uniform 512-token blocks: smaller N makes the per-matmul LDWEIGHTS
    # (~97ns, FWL off in this toolchain) stop hiding inside the matmul
    # streaming window, measured +94ns/matmul at N=256.
    blocks = []
    off = 0
    while off < C:
        size = min(NBLK, C - off)
        blocks.append((off, size))
        off += size

    with TileContext(nc) as tc:
        with (
            tc.tile_pool(name="w", bufs=1) as wpool,
            tc.tile_pool(name="xin", bufs=2) as xpool,
            tc.tile_pool(name="h", bufs=1) as hpool,
            tc.tile_pool(name="yout", bufs=2) as ypool,
            tc.tile_pool(name="ph", bufs=3, space="PSUM") as phpool,
            tc.tile_pool(name="py", bufs=4, space="PSUM") as pypool,
            tc.tile_pool(name="pw", bufs=1, space="PSUM") as pwpool,
        ):
            KH = KO // 2  # x blocks load as two half-tiles (finer DMA deps)

            def load_x_block(bi):
                xa = xpool.tile([P, KH, NBLK], dt.bfloat16, tag="xa")
                nc.sync.dma_start(xa[:], xt[bi, 0])
                xb = xpool.tile([P, KH, NBLK], dt.bfloat16, tag="xb")
                nc.sync.dma_start(xb[:], xt[bi, 1])
                return xa, xb

            def x_chunk(xts, ko):
                return xts[0][:, ko, :] if ko < KH else xts[1][:, ko - KH, :]

            # Warm the PE's HAM clock gate during the startup DMA window
            # with dummy matmuls on zeroed SBUF. The clock ramp finishes
            # after ~6-8 matmuls; the real stream is gated by xa+w1fo0
            # arrival (~13us under 8-core HBM contention), so 13 warmups
            # (~4us from ~8.3us) cover the ramp and most of the DMA window
            # without pushing the real stream past data-ready.
            warm = wpool.tile([P, NBLK], dt.bfloat16, tag="warm")
            nc.gpsimd.memset(warm[:], 0.0)
            pwarm = pwpool.tile([P, NBLK], dt.float32, tag="pw")
            NWARM = 16
            for i in range(NWARM):
                nc.tensor.matmul(
                    pwarm[:], warm[:, :P], warm[:],
                    start=(i == 0), stop=(i == NWARM - 1),
                )

            # DMA issue order is the startup critical path (each issue costs
            # ~0.6us on the sync queue): xa and w1-fo0 gate the first matmul,
            # xb is needed 4 matmuls later. b1 (tiny, needed at the first
            # gelu ~1.7us into the stream) issues from the idle gpsimd queue
            # so it doesn't take a slot in the critical sync sequence.
            xa0 = xpool.tile([P, KH, NBLK], dt.bfloat16, tag="xa")
            nc.sync.dma_start(xa0[:], xt[0, 0])

            w1sb = []
            t_ = wpool.tile([P, KO, P], dt.bfloat16, tag="w1_0")
            nc.sync.dma_start(t_[:], w1t[0])
            w1sb.append(t_)

            xb0 = xpool.tile([P, KH, NBLK], dt.bfloat16, tag="xb")
            nc.sync.dma_start(xb0[:], xt[0, 1])
            xts0 = (xa0, xb0)

            b1sb = wpool.tile([P, FO], dt.float32, tag="b1")
            nc.gpsimd.dma_start(b1sb[:], b1p[:])

            for fo in range(1, FO):
                t_ = wpool.tile([P, KO, P], dt.bfloat16, tag=f"w1_{fo}")
                nc.sync.dma_start(t_[:], w1t[fo])
                w1sb.append(t_)

            b2sb = wpool.tile([P, D], dt.float32, tag="b2")
            nc.sync.dma_start(b2sb[:], b2r[:])
            scsb = wpool.tile([P, C // P], dt.float32, tag="sc")
            nc.sync.dma_start(scsb[:], scp[:])

            # w2 in 4 chunks: a single 8MB DMA's completion gated the whole
            # Y phase (all 8 cores contend for HBM in the first ~70us and a
            # late w2 stalls the PE ~5us + re-throttles the clock). Chunk 0
            # (2MB) lands ~30us before the first Y matmul needs it, and later
            # chunks arrive while Y streams earlier fo's.
            W2C = 8
            FOC = FO // W2C
            w2_r = w2.rearrange("(fo p) d -> p fo d", p=P)
            w2sb = []
            for c in range(W2C):
                t_ = wpool.tile([P, FOC, D], dt.bfloat16, tag=f"w2_{c}")
                nc.sync.dma_start(t_[:], w2_r[:, c * FOC:(c + 1) * FOC])
                w2sb.append(t_)

            for bi, (n_off, n_size) in enumerate(blocks):
                xts = xts0 if bi == 0 else load_x_block(bi)

                # H^T[f, t] = sum_d w1[d, f] * x^T[d, t], then gelu(+b1).
                htile = hpool.tile([P, FO, NBLK], dt.bfloat16, tag="h")
                for fo in range(FO):
                    ph = phpool.tile([P, NBLK], dt.float32, tag="ph")
                    for ko in range(KO):
                        nc.tensor.matmul(
                            ph[:, :n_size],
                            w1sb[fo][:, ko, :],
                            x_chunk(xts, ko)[:, :n_size],
                            start=(ko == 0),
                            stop=(ko == KO - 1),
                        )
                    nc.scalar.activation(
                        htile[:, fo, :n_size], ph[:, :n_size], GELU,
                        bias=b1sb[:, fo:fo + 1], scale=1.0,
                    )

                # Y[t, d] = sum_f H[t, f] * w2[f, d]; scale per token.
                for tb in range(n_size // P):
                    tbg = (n_off + tb * P) // P
                    ytile = ypool.tile([P, D], dt.bfloat16, tag="y")
                    # dn-outer: the d-half 0 epilogue (scale, store) overlaps
                    # the d-half 1 matmuls, so little epilogue trails the
                    # very last matmul of the kernel.
                    for dn in range(DN):
                        dsl = slice(dn * NBLK, (dn + 1) * NBLK)
                        py = pypool.tile([P, NBLK], dt.float32, tag="py")
                        # Seed PSUM with b2 (vector write), accumulate
                        # matmuls on top with start=False: removes the bias
                        # add from the post-matmul critical path.
                        nc.vector.tensor_copy(py[:], b2sb[:, dsl])
                        for fo in range(FO):
                            nc.tensor.matmul(
                                py[:],
                                htile[:, fo, tb * P:(tb + 1) * P],
                                w2sb[fo // FOC][:, fo % FOC,
                                                dn * NBLK:(dn + 1) * NBLK],
                                start=False,
                                stop=(fo == FO - 1),
                                # start=False with a DVE-seeded bank: the
                                # sim's group bookkeeping never sees a start.
                                skip_group_check=True,
                            )
                        rows = slice(n_off + tb * P, n_off + (tb + 1) * P)
                        nc.vector.tensor_scalar_mul(
                            ytile[:, dsl], py[:], scsb[:, tbg:tbg + 1]
                        )
                        nc.sync.dma_start(y[rows, dsl], ytile[:, dsl])
    nc.compile()
    return nc


def _route(flat, gate_w, gate_b):
    """fp32 gate matching the reference: softmax, top-2, renormalize."""
    logits = flat @ gate_w + gate_b
    m = logits.max(axis=1, keepdims=True)
    p = np.exp(logits - m, dtype=np.float32)
    probs = p / p.sum(axis=1, keepdims=True)
    ti = np.argsort(-probs, axis=1, kind="stable")[:, :TOPK]
    tp = np.take_along_axis(probs, ti, axis=1)
    sw = tp / (tp.sum(axis=1, keepdims=True) + np.float32(1e-9))
    return ti.astype(np.int64), sw.astype(np.float32)


def _dispatch(ti):
    """Slot assignment: (token, k) pair -> (expert, position-in-expert)."""
    Tn = ti.shape[0]
    flat_e = ti.ravel()
    order = np.argsort(flat_e, kind="stable")
    cnt = np.bincount(flat_e, minlength=E)
    starts = np.concatenate([[0], np.cumsum(cnt)[:-1]])
    ranks = np.arange(Tn * TOPK) - starts[flat_e[order]]
    pos = np.empty(Tn * TOPK, np.int64)
    pos[order] = ranks
    return flat_e, pos, cnt, starts, order


def _gelu_exact(v):
    try:
        from scipy.special import erf
        return 0.5 * v * (1.0 + erf(v / np.sqrt(2.0)))
    except ImportError:  # tanh approximation fallback (overflow tokens only)
        return 0.5 * v * (1.0 + np.tanh(
            0.7978845608028654 * (v + 0.044715 * v ** 3)))


def kernel(**inputs) -> np.ndarray:
    global LAST
    x = np.asarray(inputs["x"], np.float32)
    gate_w = np.asarray(inputs["gate_w"], np.float32)
    gate_b = np.asarray(inputs["gate_b"], np.float32)
    w1 = np.asarray(inputs["w1"], np.float32)
    b1 = np.asarray(inputs["b1"], np.float32)
    w2 = np.asarray(inputs["w2"], np.float32)
    b2 = np.asarray(inputs["b2"], np.float32)

    B, S, D_ = x.shape
    flat = x.reshape(-1, D_)
    Tn = flat.shape[0]

    ti, sw = _route(flat, gate_w, gate_b)
    flat_e, pos, cnt, starts, order = _dispatch(ti)

    # Capacity factor 1.0: each core processes exactly T*K/E token slots (the
    # SPMD program is uniform, so every core pays the max expert's cost —
    # capping at the mean keeps the device critical path balanced). The few
    # overflow tokens of the hottest experts are combined on the host in fp32.
    cap = (Tn * TOPK // E + P - 1) // P * P
    C = ((int(cnt.max()) + P - 1) // P) * P
    C = max(min(C, cap), P)

    xT_bf = np.ascontiguousarray(flat.T).astype(_BF16)  # [D, T]
    sw_flat = sw.ravel()

    in_maps = []
    overflow = []
    for e in range(E):
        pairs_all = order[starts[e]:starts[e] + cnt[e]]
        pairs = pairs_all[:C]
        if cnt[e] > C:
            overflow.append((e, pairs_all[C:]))
        n_e = len(pairs)
        toks = pairs // TOPK
        xt_e = np.zeros((D, C), _BF16)
        xt_e[:, :n_e] = xT_bf[:, toks]
        # Pre-tile to the SBUF image [block, half, p, ko_local, tok] so each
        # x DMA is one contiguous 4KB line per partition.
        NB = C // NBLK
        KH = (D // P) // 2
        xt_tiled = np.ascontiguousarray(
            xt_e.reshape(2, KH, P, NB, NBLK).transpose(3, 0, 2, 1, 4)
        )
        sc_e = np.zeros((C,), np.float32)
        sc_e[:n_e] = sw_flat[pairs]
        KO, FO = D // P, F // P
        w1_tiled = np.ascontiguousarray(
            w1[e].astype(_BF16).reshape(KO, P, FO, P).transpose(2, 1, 0, 3)
        )
        in_maps.append({
            "xt": xt_tiled,
            "w1t": w1_tiled,
            "w2": w2[e].astype(_BF16),
            "b1p": np.ascontiguousarray(b1[e].reshape(F // P, P).T),
            "b2r": np.ascontiguousarray(
                np.broadcast_to(b2[e], (P, D))
            ).astype(np.float32),
            "scp": np.ascontiguousarray(sc_e.reshape(C // P, P).T),
        })

    nc = _nc_cache.get(C)
    if nc is None:
        nc = _build_moe_core(C)
        _nc_cache[C] = nc

    LAST = run_bass_kernel_spmd(nc, in_maps, core_ids=list(range(E)))
    Yall = np.stack([
        np.asarray(LAST.results[i]["y"]).astype(np.float32) for i in range(E)
    ])

    # Combine: device slots via two gathers; host fp32 FFN for overflow.
    in_cap = pos < C
    contrib = np.zeros((Tn * TOPK, D_), np.float32)
    idx = np.nonzero(in_cap)[0]
    contrib[idx] = Yall[flat_e[idx], pos[idx]]
    out = contrib[0::TOPK] + contrib[1::TOPK]
    for e, over in overflow:
        toks = over // TOPK
        h = _gelu_exact(flat[toks] @ w1[e] + b1[e])
        y_e = h @ w2[e] + b2[e]
        out[toks] += sw_flat[over][:, None] * y_e
    return out.reshape(B, S, D_).astype(np.float32)

